# revision 1
# baseline (speedup 1.0000x reference)
"""Multi-head attention (B=4, S=2048, D=1024, H=16, Dh=64) on 8 TRN2 NeuronCores.

Sharding: core c handles batch b = c // 2 and head group g = c % 2 (8 heads
each).  Every core computes Q/K/V projections for its batch+heads, the
attention for those heads, and a *partial* output projection (its heads'
slice of Wo).  The host sums the two partials per batch while unsharding —
the tensor-parallel all-reduce on the output, done during gather.

Per-core dataflow (all matmuls bf16 operands, fp32 PSUM accumulation):
  - host supplies X^T [D, S] per input so the contraction dim is always on
    SBUF partitions; no on-device transposes anywhere.
  - Q^T, K^T stored [hk, S] (hk = 8 heads * 64); V stored [t, hk] with an
    extra ones column per head.
  - logits^T[t, f] = (K^T_h).T @ Q^T_h  (K=64; the two heads of an SBUF
    partition-tile run concurrently via PE row tiling).
  - expS = Exp(0.125 * logits^T) on ScalarE (softmax scale folded into the
    activation's free affine; no max subtraction needed: logits ~ N(0,1)).
  - ctx^T/denna = (V_ones).T @ expS accumulated over t: rows 0..63 are the
    unnormalized ctx^T, row 64 is the softmax denominator — for free.
  - normalization deferred: denominators collected into one [16, CW] tile,
    one batched DVE reciprocal, broadcast across partitions with a tiny
    constant selection matmul on the PE, one tensor_mul per chunk.
  - out_part[f, d] accumulated over the four 128-row chunks of ctx^T.
"""

import sys

sys.path.insert(0, "/opt/trn_rl_repo")

import numpy as np
import ml_dtypes

BF = ml_dtypes.bfloat16

# Problem geometry (hardcoded; the harness always calls with these shapes).
B, S, D, H, Dh = 4, 2048, 1024, 16, 64
N_CORES = 8
H_LOC = H // 2          # heads per core
HK = H_LOC * Dh         # 512


class Cfg:
    def __init__(self, S=S, D=D, hloc=H_LOC, Dh=Dh):
        P = 128
        self.S, self.D, self.hloc, self.Dh = S, D, hloc, Dh
        self.P = P
        self.hk = hloc * Dh
        assert self.hk % P == 0 and self.hk <= 512
        self.MJ = self.hk // P        # partition tiles of hk (2 heads each)
        self.J = hloc // 2
        assert self.MJ == self.J
        self.DC = D // P              # contraction chunks for projections
        self.TT = S // P              # t (key) tiles
        self.CW = min(1024, S)        # f-chunk width
        self.NCC = S // self.CW       # f-chunks
        self.NB = self.CW // 512      # PSUM banks per f-chunk
        self.ND = (D + 511) // 512    # 512-wide slices of D
        self.scale = float(Dh) ** -0.5


def make_sel(cfg):
    """sel[r, (j*NCC+cc)*P + p] = 1 where r == (2j + p//64)*NCC + cc.

    Used as matmul lhsT to broadcast reciprocal-denominator rows across the
    64 partitions of each head's ctx^T slice."""
    rows = cfg.hloc * cfg.NCC
    sel = np.zeros((rows, cfg.J * cfg.NCC * cfg.P), np.float32)
    for j in range(cfg.J):
        for cc in range(cfg.NCC):
            base = (j * cfg.NCC + cc) * cfg.P
            for p in range(cfg.P):
                sel[(2 * j + p // 64) * cfg.NCC + cc, base + p] = 1.0
    return sel


def build_nc(cfg):
    import concourse.bass as bass
    import concourse.mybir as mybir
    import concourse.tile as tile
    from concourse import bacc
    from concourse.bass import ds, ts
    from contextlib import ExitStack

    FP32 = mybir.dt.float32
    BF16 = mybir.dt.bfloat16
    EXP = mybir.ActivationFunctionType.Exp

    P, Dh_, hloc = cfg.P, cfg.Dh, cfg.hloc
    S_, D_, hk = cfg.S, cfg.D, cfg.hk
    J, MJ, DC, TT, CW, NCC, NB, ND = (
        cfg.J, cfg.MJ, cfg.DC, cfg.TT, cfg.CW, cfg.NCC, cfg.NB, cfg.ND)
    selrows = hloc * NCC

    nc = bacc.Bacc("TRN2")
    xq = nc.declare_dram_parameter("xq_t", [D_, S_], BF16, isOutput=False)
    xk = nc.declare_dram_parameter("xk_t", [D_, S_], BF16, isOutput=False)
    xv = nc.declare_dram_parameter("xv_t", [D_, S_], BF16, isOutput=False)
    wq = nc.declare_dram_parameter("wq", [D_, hk], BF16, isOutput=False)
    wk = nc.declare_dram_parameter("wk", [D_, hk], BF16, isOutput=False)
    wv = nc.declare_dram_parameter("wv", [D_, hk], BF16, isOutput=False)
    wo = nc.declare_dram_parameter("wo", [hk, D_], BF16, isOutput=False)
    out = nc.declare_dram_parameter("out_part", [S_, D_], FP32, isOutput=True)

    with tile.TileContext(nc) as tc, ExitStack() as ctx:
        singles = ctx.enter_context(tc.tile_pool(name="singles", bufs=1))

        # ---- persistent SBUF tensors -------------------------------------
        wq_sb = singles.tile([P, DC, hk], BF16, tag="wq", name="wq")
        wk_sb = singles.tile([P, DC, hk], BF16, tag="wk", name="wk")
        wv_sb = singles.tile([P, DC, hk], BF16, tag="wv", name="wv")
        wo_sb = singles.tile([P, MJ, D_], BF16, tag="wo", name="wo")
        qT = [singles.tile([P, S_], BF16, tag=f"qT{j}", name=f"qT{j}") for j in range(MJ)]
        kT = [singles.tile([P, S_], BF16, tag=f"kT{j}", name=f"kT{j}") for j in range(MJ)]
        ct = [singles.tile([P, S_], BF16, tag=f"ct{j}", name=f"ct{j}") for j in range(MJ)]
        vt = [singles.tile([P, hloc, Dh_ + 1], BF16, tag=f"vt{m}", name=f"vt{m}")
              for m in range(TT)]

        # wq chunk 0 first: the very first matmul needs only wq[dc=0] + the
        # first xq chunk
        wq_r = wq[:, :].rearrange("(a p) n -> p a n", p=P)
        for dc in range(DC):
            nc.sync.dma_start(out=wq_sb[:, dc, :], in_=wq_r[:, dc, :])

        # ---- phase P: projections ----------------------------------------
        with tc.tile_pool(name="xin", bufs=2) as xpool, \
             tc.tile_pool(name="psumP", bufs=2, space="PSUM") as pps:

            def load_xt(src):
                # one DMA per contraction chunk so the first matmuls can
                # start as soon as chunk 0 lands
                xt = xpool.tile([P, DC, S_], BF16, tag="xt", name="xt")
                src_r = src[:, :].rearrange("(a p) s -> p a s", p=P)
                for dc in range(DC):
                    nc.sync.dma_start(out=xt[:, dc, :], in_=src_r[:, dc, :])
                return xt

            def project_T(xt, w_sb, dst):
                # dst[j][hk_row, f] = sum_d w[d, hk_row] * x^T[d, f]
                for j in range(MJ):
                    for cc in range(NCC):
                        ps = pps.tile([P, CW], FP32, tag="psq", name="psq")
                        for dc in range(DC):
                            for nb in range(NB):
                                nc.tensor.matmul(
                                    ps[:, ts(nb, 512)],
                                    lhsT=w_sb[:, dc, ts(j, P)],
                                    rhs=xt[:, dc, ds(cc * CW + nb * 512, 512)],
                                    start=(dc == 0), stop=(dc == DC - 1))
                        nc.vector.tensor_copy(out=dst[j][:, ds(cc * CW, CW)],
                                              in_=ps)

            xt = load_xt(xq)
            nc.sync.dma_start(out=wk_sb,
                              in_=wk[:, :].rearrange("(a p) n -> p a n", p=P))
            project_T(xt, wq_sb, qT)
            xt = load_xt(xk)
            nc.sync.dma_start(out=wv_sb,
                              in_=wv[:, :].rearrange("(a p) n -> p a n", p=P))
            project_T(xt, wk_sb, kT)
            xt = load_xt(xv)
            nc.sync.dma_start(out=wo_sb,
                              in_=wo[:, :].rearrange("(j p) d -> p j d", p=P))
            # V[t, hk] tiles + ones column per head
            for m in range(TT):
                ps = pps.tile([P, hk], FP32, tag="psv", name="psv")
                for dc in range(DC):
                    nc.tensor.matmul(ps, lhsT=xt[:, dc, ts(m, P)],
                                     rhs=wv_sb[:, dc, :],
                                     start=(dc == 0), stop=(dc == DC - 1))
                nc.vector.tensor_copy(
                    out=vt[m][:, :, 0:Dh_],
                    in_=ps.rearrange("p (h k) -> p h k", h=hloc))
                nc.vector.memset(vt[m][:, :, Dh_:Dh_ + 1], 1.0)

        # ---- phase D: attention (+ incremental softmax normalization) ----
        with tc.tile_pool(name="psumL", bufs=1, space="PSUM") as ppl, \
             tc.tile_pool(name="psumC", bufs=1, space="PSUM") as ppc, \
             tc.tile_pool(name="expp", bufs=4) as epool, \
             tc.tile_pool(name="rbc", bufs=2) as rpool, \
             tc.tile_pool(name="stage", bufs=2) as stpool:

            def logits_one(j, cc, m, o, tag):
                pl = ppl.tile([P, CW], FP32, tag=tag, name=tag)
                for nb in range(NB):
                    nc.tensor.matmul(
                        pl[:, ts(nb, 512)],
                        lhsT=kT[j][o:o + 64, ts(m, P)],
                        rhs=qT[j][o:o + 64, ds(cc * CW + nb * 512, 512)],
                        start=True, stop=True)
                return pl

            def ctx_one(pc, e, m, h):
                for nb in range(NB):
                    nc.tensor.matmul(
                        pc[:, ts(nb, 512)],
                        lhsT=vt[m][:, h, 0:Dh_ + 1],
                        rhs=e[:, ts(nb, 512)],
                        start=(m == 0), stop=(m == TT - 1))

            for j in range(J):
                hA, hB = 2 * j, 2 * j + 1
                for cc in range(NCC):
                    pcA = ppc.tile([Dh_ + 1, CW], FP32, tag="pcA", name="pcA")
                    pcB = ppc.tile([Dh_ + 1, CW], FP32, tag="pcB", name="pcB")
                    plA = logits_one(j, cc, 0, 0, "plA")
                    plB = logits_one(j, cc, 0, 64, "plB")
                    for m in range(TT):
                        eA = epool.tile([P, CW], BF16, tag="eA", name="eA")
                        nc.scalar.activation(out=eA, in_=plA, func=EXP,
                                             scale=cfg.scale)
                        eB = epool.tile([P, CW], BF16, tag="eB", name="eB")
                        nc.scalar.activation(out=eB, in_=plB, func=EXP,
                                             scale=cfg.scale)
                        # software pipeline, interleaved so PE idle is split
                        # into short even gaps: lA(m+1), cA(m), lB(m+1), cB(m)
                        if m + 1 < TT:
                            plA = logits_one(j, cc, m + 1, 0, "plA")
                        ctx_one(pcA, eA, m, hA)
                        if m + 1 < TT:
                            plB = logits_one(j, cc, m + 1, 64, "plB")
                        ctx_one(pcB, eB, m, hB)
                    # --- epilogue: softmax normalization fused into the
                    # PSUM drain.  HW constraints (micro-tested):
                    # reciprocal_approx_fast needs base-0 flat 2D APs, and
                    # gpsimd partition_broadcast needs src on partition 0 /
                    # dst starting at partition 0.  So: lane-aligned copy of
                    # the PSUM denominator row, DMA partition-shift to 0,
                    # fast reciprocal, broadcast; head B is normalized
                    # BEFORE its partition-shift DMA so all DVE ops stay
                    # base-0.
                    stA = stpool.tile([Dh_ + 1, CW], FP32, tag="stA", name="stA")
                    nc.vector.tensor_copy(out=stA[Dh_:Dh_ + 1, :],
                                          in_=pcA[Dh_:Dh_ + 1, :])
                    stB = stpool.tile([Dh_ + 1, CW], FP32, tag="stB", name="stB")
                    nc.vector.tensor_copy(out=stB[Dh_:Dh_ + 1, :],
                                          in_=pcB[Dh_:Dh_ + 1, :])
                    d0A = rpool.tile([1, CW], FP32, tag="d0A", name="d0A")
                    nc.sync.dma_start(out=d0A, in_=stA[Dh_:Dh_ + 1, :])
                    d0B = rpool.tile([1, CW], FP32, tag="d0B", name="d0B")
                    nc.sync.dma_start(out=d0B, in_=stB[Dh_:Dh_ + 1, :])
                    rA = rpool.tile([1, CW], FP32, tag="rA", name="rA")
                    nc.vector.reciprocal_approx_fast(out=rA, in_=d0A)
                    rB = rpool.tile([1, CW], FP32, tag="rB", name="rB")
                    nc.vector.reciprocal_approx_fast(out=rB, in_=d0B)
                    rbA = rpool.tile([Dh_, CW], FP32, tag="rbA", name="rbA")
                    nc.gpsimd.partition_broadcast(rbA, rA, channels=Dh_)
                    rbB = rpool.tile([Dh_, CW], FP32, tag="rbB", name="rbB")
                    nc.gpsimd.partition_broadcast(rbB, rB, channels=Dh_)
                    # head A: normalize straight into ct (partitions 0..63)
                    nc.vector.tensor_mul(out=ct[j][0:64, ds(cc * CW, CW)],
                                         in0=pcA[0:Dh_, :], in1=rbA)
                    # head B: normalize into a base-0 temp, then DMA-shift to
                    # partitions 64..127.
                    tmB = stpool.tile([Dh_, CW], BF16, tag="tmB", name="tmB")
                    nc.vector.tensor_mul(out=tmB, in0=pcB[0:Dh_, :], in1=rbB)
                    nc.sync.dma_start(out=ct[j][64:128, ds(cc * CW, CW)],
                                      in_=tmB)

        # ---- phase E: output projection ----------------------------------
        with tc.tile_pool(name="psumO", bufs=2, space="PSUM") as ppo, \
             tc.tile_pool(name="outb", bufs=3) as obpool:
            ndw = min(512, D_)
            for ft in range(TT):
                po = ppo.tile([P, D_], FP32, tag="po", name="po")
                for j in range(MJ):
                    for nd in range(D_ // ndw):
                        nc.tensor.matmul(
                            po[:, ts(nd, ndw)],
                            lhsT=ct[j][:, ts(ft, P)],
                            rhs=wo_sb[:, j, ts(nd, ndw)],
                            start=(j == 0), stop=(j == MJ - 1))
                ob = obpool.tile([P, D_], FP32, tag="ob", name="ob")
                nc.vector.tensor_copy(out=ob, in_=po)
                nc.sync.dma_start(out=out[ts(ft, P), :], in_=ob)

    nc.compile()
    return nc


def shard_inputs(cfg, query_input, key_input, value_input, Wq, Wk, Wv, Wo):
    """Per-core input maps: core c -> batch c//2, head group c%2."""
    hloc = cfg.hloc
    in_maps = []
    for c in range(N_CORES):
        b, g = c // 2, c % 2
        hs = slice(g * hloc, (g + 1) * hloc)
        in_maps.append({
            "xq_t": np.ascontiguousarray(query_input[b].T).astype(BF),
            "xk_t": np.ascontiguousarray(key_input[b].T).astype(BF),
            "xv_t": np.ascontiguousarray(value_input[b].T).astype(BF),
            "wq": np.ascontiguousarray(Wq[:, hs, :]).reshape(cfg.D, cfg.hk).astype(BF),
            "wk": np.ascontiguousarray(Wk[:, hs, :]).reshape(cfg.D, cfg.hk).astype(BF),
            "wv": np.ascontiguousarray(Wv[:, hs, :]).reshape(cfg.D, cfg.hk).astype(BF),
            "wo": np.ascontiguousarray(Wo[hs]).reshape(cfg.hk, cfg.D).astype(BF),
        })
    return in_maps


_nc_cache = {}


def _get_nc(cfg):
    key = (cfg.S, cfg.D, cfg.hloc, cfg.Dh)
    if key not in _nc_cache:
        _nc_cache[key] = build_nc(cfg)
    return _nc_cache[key]


def run_spmd(inputs, trace=False, trace_cores=None):
    """Run the 8-core SPMD kernel; returns (output [B,S,D] fp32, BassKernelResults)."""
    from concourse.bass_utils import run_bass_kernel_spmd

    cfg = Cfg()
    nc = _get_nc(cfg)
    in_maps = shard_inputs(cfg, **{k: np.asarray(v) for k, v in inputs.items()})
    res = run_bass_kernel_spmd(nc, in_maps, list(range(N_CORES)),
                               trace=trace, trace_cores=trace_cores)
    out = np.empty((B, S, D), np.float32)
    for b in range(B):
        out[b] = res.results[2 * b]["out_part"] + res.results[2 * b + 1]["out_part"]
    return out, res


def kernel(**inputs):
    out, _ = run_spmd(inputs)
    return out



# revision 3
# speedup vs baseline: 1.0610x; 1.0610x over previous
"""Multi-head attention (B=4, S=2048, D=1024, H=16, Dh=64) on 8 TRN2 NeuronCores.

Sharding: core c handles batch b = c // 2 and head group g = c % 2 (8 heads
each).  Every core computes Q/K/V projections for its batch+heads, the
attention for those heads, and a *partial* output projection (its heads'
slice of Wo).  The host sums the two partials per batch while unsharding —
the tensor-parallel all-reduce on the output, done during gather.

v2 restructure (all bf16 matmul operands, fp32 PSUM accumulation):
  - logits^T[t,f] per head pair via two K=64 matmuls issued back-to-back so
    they run CONCURRENTLY on disjoint PE row groups (rows 0-63 / 64-127).
  - one ScalarE Exp instruction per (m, head-pair): reads the [128, 2, 512]
    logits PSUM tile flat (FD=1024).  ScalarE does nothing else; it is the
    phase-D pacing engine (~255us of exp is the per-engine floor).
  - pl (logits PSUM) double-buffered so the m+1 logits pair never waits on
    the exp of m.
  - vt value tiles are padded to 128 columns with ONES in cols 64..127: the
    ctx matmul then yields ctx^T rows 0..63 and 64 replicated denominator
    rows 64..127 in one [128, 512] output — full PE column utilization, a
    128-wide stationary operand (fast-weight-load eligible), and a free
    denominator "broadcast".
  - softmax epilogue per (j, cc): lane-aligned [64,512] copy of the
    replicated denominator rows, partition-shift DMA to base-0, one
    [64,512] reciprocal_approx_fast, two [64,512] tensor_muls (head B via a
    base-0 temp + partition-shift DMA, as DVE lanes are hardwired per
    partition).
  - q/k projections for head pair j+1 (and V tiles past the warmup) are
    emitted as FILLER work inside head pair j's attention m-loop, hiding the
    projection phase in the PE bubbles of the exp-paced attention.  x^T
    inputs are DMAed from DRAM per head pair in S/2 halves (re-loaded for
    every j) to keep SBUF small — DMA bandwidth has large headroom.
"""

import sys

sys.path.insert(0, "/opt/trn_rl_repo")

import numpy as np
import ml_dtypes

BF = ml_dtypes.bfloat16

# Problem geometry (hardcoded; the harness always calls with these shapes).
B, S, D, H, Dh = 4, 2048, 1024, 16, 64
N_CORES = 8
H_LOC = H // 2          # heads per core
HK = H_LOC * Dh         # 512


class Cfg:
    def __init__(self, S=S, D=D, hloc=H_LOC, Dh=Dh):
        P = 128
        self.S, self.D, self.hloc, self.Dh = S, D, hloc, Dh
        self.P = P
        self.hk = hloc * Dh           # 512
        self.J = hloc // 2            # head pairs = 4
        self.DC = D // P              # contraction chunks = 8
        self.TT = S // P              # t (key) tiles = 16
        self.CW = 512                 # f-chunk width (one PSUM bank)
        self.NCC = S // self.CW       # f-chunks = 4
        self.scale = float(Dh) ** -0.5


def build_nc(cfg):
    import concourse.bass as bass
    import concourse.mybir as mybir
    import concourse.tile as tile
    from concourse import bacc
    from concourse.bass import ds, ts
    from contextlib import ExitStack

    FP32 = mybir.dt.float32
    BF16 = mybir.dt.bfloat16
    EXP = mybir.ActivationFunctionType.Exp

    P, Dh_, hloc = cfg.P, cfg.Dh, cfg.hloc
    S_, D_, hk = cfg.S, cfg.D, cfg.hk
    J, DC, TT, CW, NCC = cfg.J, cfg.DC, cfg.TT, cfg.CW, cfg.NCC
    SH = S_ // 2                       # x half width

    nc = bacc.Bacc("TRN2")
    xq = nc.declare_dram_parameter("xq_t", [D_, S_], BF16, isOutput=False)
    xk = nc.declare_dram_parameter("xk_t", [D_, S_], BF16, isOutput=False)
    xv = nc.declare_dram_parameter("xv_t", [D_, S_], BF16, isOutput=False)
    wq = nc.declare_dram_parameter("wq", [D_, hk], BF16, isOutput=False)
    wk = nc.declare_dram_parameter("wk", [D_, hk], BF16, isOutput=False)
    wv = nc.declare_dram_parameter("wv", [D_, hk], BF16, isOutput=False)
    wo = nc.declare_dram_parameter("wo", [hk, D_], BF16, isOutput=False)
    out = nc.declare_dram_parameter("out_part", [S_, D_], FP32, isOutput=True)

    xs = {"q": xq, "k": xk, "v": xv}

    with tile.TileContext(nc) as tc, ExitStack() as ctx:
        singles = ctx.enter_context(tc.tile_pool(name="singles", bufs=1))

        # ---- persistent SBUF tensors -------------------------------------
        wq_sb = singles.tile([P, DC, hk], BF16, tag="wq", name="wq")
        wk_sb = singles.tile([P, DC, hk], BF16, tag="wk", name="wk")
        wv_sb = singles.tile([P, DC, hk], BF16, tag="wv", name="wv")
        wo_sb = singles.tile([P, J, D_], BF16, tag="wo", name="wo")
        qT = [singles.tile([P, S_], BF16, tag=f"qT{j}", name=f"qT{j}") for j in range(J)]
        kT = [singles.tile([P, S_], BF16, tag=f"kT{j}", name=f"kT{j}") for j in range(J)]
        ct = [singles.tile([P, S_], BF16, tag=f"ct{j}", name=f"ct{j}") for j in range(J)]
        # vt[m]: [t, head, 128]; cols 0..63 = V, cols 64..127 = 1.0 so the
        # ctx matmul replicates the softmax denominator on rows 64..127.
        vt = [singles.tile([P, hloc, P], BF16, tag=f"vt{m}", name=f"vt{m}")
              for m in range(TT)]

        w_sbs = {"q": wq_sb, "k": wk_sb, "v": wv_sb}

        # weight DMAs up front (wq chunk 0 first: the very first projection
        # matmul needs only wq[dc=0] + the first xq chunk)
        for w_dram, w_sb in ((wq, wq_sb), (wk, wk_sb), (wv, wv_sb)):
            w_r = w_dram[:, :].rearrange("(a p) n -> p a n", p=P)
            for dc in range(DC):
                nc.sync.dma_start(out=w_sb[:, dc, :], in_=w_r[:, dc, :])
        nc.sync.dma_start(out=wo_sb,
                          in_=wo[:, :].rearrange("(j p) d -> p j d", p=P))

        with tc.tile_pool(name="xv_in", bufs=1) as xvpool, \
             tc.tile_pool(name="xqk_in", bufs=2) as xqkpool, \
             tc.tile_pool(name="psumP", bufs=2, space="PSUM") as pps, \
             tc.tile_pool(name="psumL", bufs=2, space="PSUM") as ppl, \
             tc.tile_pool(name="psumC", bufs=1, space="PSUM") as ppc, \
             tc.tile_pool(name="expp", bufs=4) as epool, \
             tc.tile_pool(name="rbc", bufs=2) as rpool, \
             tc.tile_pool(name="stage", bufs=2) as stpool:

            # ---- filler machinery: small closures emitted inside the
            # attention m-loop so projection work fills PE bubbles ----------
            def x_dma_fillers(which, half):
                """Closures DMAing one half of an x^T input into a new tile."""
                pool = xvpool if which == "v" else xqkpool
                xt = pool.tile([P, DC, SH], BF16, tag=f"xt_{which != 'v'}",
                               name=f"xt{which}{half}")
                src_r = xs[which][:, :].rearrange("(a p) s -> p a s", p=P)

                def mk(dc):
                    def emit():
                        nc.sync.dma_start(out=xt[:, dc, :],
                                          in_=src_r[:, dc, ds(half * SH, SH)])
                    return emit
                return xt, [mk(dc) for dc in range(DC)]

            def qk_proj_fillers(which, j, xt_half, half):
                """Matmul+drain closures projecting one x half -> qT/kT[j]."""
                dst = qT[j] if which == "q" else kT[j]
                w_sb = w_sbs[which]
                fillers = []
                for w in range(SH // CW):
                    holder = []

                    def mk_mm(dc, w=w, holder=holder):
                        def emit():
                            if dc == 0:
                                holder.append(pps.tile([P, CW], FP32,
                                                       tag="psP", name="psP"))
                            nc.tensor.matmul(
                                holder[0],
                                lhsT=w_sb[:, dc, ts(j, P)],
                                rhs=xt_half[:, dc, ds(w * CW, CW)],
                                start=(dc == 0), stop=(dc == DC - 1))
                        return emit

                    def mk_drain(w=w, holder=holder):
                        def emit():
                            nc.vector.tensor_copy(
                                out=dst[:, ds(half * SH + w * CW, CW)],
                                in_=holder[0])
                        return emit

                    fillers += [mk_mm(dc) for dc in range(DC)]
                    fillers.append(mk_drain())
                return fillers

            def v_proj_fillers(m, xt_half):
                """Matmul+drain closures producing the padded vt[m] tile."""
                holder = []

                def mk_mm(dc):
                    def emit():
                        if dc == 0:
                            holder.append(pps.tile([P, hk], FP32,
                                                   tag="psP", name="psP"))
                        nc.tensor.matmul(holder[0],
                                         lhsT=xt_half[:, dc, ts(m % 8, P)],
                                         rhs=wv_sb[:, dc, :],
                                         start=(dc == 0), stop=(dc == DC - 1))
                    return emit

                def drain():
                    nc.vector.tensor_copy(
                        out=vt[m][:, :, 0:Dh_],
                        in_=holder[0].rearrange("p (h k) -> p h k", h=hloc))
                    nc.vector.memset(vt[m][:, :, Dh_:P], 1.0)

                return [mk_mm(dc) for dc in range(DC)] + [drain]

            # ---- attention ------------------------------------------------
            def lpair(j, cc, m, pl):
                # two K=64 logits matmuls on disjoint PE row groups
                for hh in range(2):
                    nc.tensor.matmul(
                        pl[:, hh, :],
                        lhsT=kT[j][hh * 64:(hh + 1) * 64, ts(m, P)],
                        rhs=qT[j][hh * 64:(hh + 1) * 64, ds(cc * CW, CW)],
                        start=True, stop=True)

            def attention_block(j, fillers, pre_ctx_quota):
                """One head pair's attention.  fillers are drained ~evenly
                across the NCC*TT m-iterations; pre_ctx_quota of them run
                BEFORE each iteration's ctx matmuls (so vt tiles produced by
                fillers stay ahead of this block's own consumption)."""
                n_iter = NCC * TT
                total = len(fillers)
                state = {"done": 0}

                def emit_fillers(n):
                    for _ in range(max(0, n)):
                        if fillers:
                            fillers.pop(0)()
                            state["done"] += 1

                it = 0
                for cc in range(NCC):
                    pcA = ppc.tile([P, CW], FP32, tag="pcA", name="pcA")
                    pcB = ppc.tile([P, CW], FP32, tag="pcB", name="pcB")
                    pl = ppl.tile([P, 2, CW], FP32, tag="pl", name="pl")
                    lpair(j, cc, 0, pl)
                    for m in range(TT):
                        e = epool.tile([P, 2, CW], BF16, tag="e", name="e")
                        nc.scalar.activation(out=e, in_=pl, func=EXP,
                                             scale=cfg.scale)
                        if m + 1 < TT:
                            pl = ppl.tile([P, 2, CW], FP32, tag="pl", name="pl")
                            lpair(j, cc, m + 1, pl)
                        emit_fillers(pre_ctx_quota)
                        for hh in range(2):
                            nc.tensor.matmul(
                                (pcA, pcB)[hh],
                                lhsT=vt[m][:, 2 * j + hh, :],
                                rhs=e[:, hh, :],
                                start=(m == 0), stop=(m == TT - 1))
                        it += 1
                        emit_fillers(it * total // n_iter - state["done"])
                    # ---- softmax epilogue for (j, cc) ----------------------
                    stA = stpool.tile([P, CW], FP32, tag="stA", name="stA")
                    nc.vector.tensor_copy(out=stA[64:128, :], in_=pcA[64:128, :])
                    stB = stpool.tile([P, CW], FP32, tag="stB", name="stB")
                    nc.vector.tensor_copy(out=stB[64:128, :], in_=pcB[64:128, :])
                    dA = rpool.tile([64, CW], FP32, tag="dA", name="dA")
                    nc.sync.dma_start(out=dA, in_=stA[64:128, :])
                    dB = rpool.tile([64, CW], FP32, tag="dB", name="dB")
                    nc.sync.dma_start(out=dB, in_=stB[64:128, :])
                    rA = rpool.tile([64, CW], FP32, tag="rA", name="rA")
                    nc.vector.reciprocal_approx_fast(out=rA, in_=dA)
                    rB = rpool.tile([64, CW], FP32, tag="rB", name="rB")
                    nc.vector.reciprocal_approx_fast(out=rB, in_=dB)
                    nc.vector.tensor_mul(out=ct[j][0:64, ds(cc * CW, CW)],
                                         in0=pcA[0:Dh_, :], in1=rA)
                    tmB = stpool.tile([64, CW], BF16, tag="tmB", name="tmB")
                    nc.vector.tensor_mul(out=tmB, in0=pcB[0:Dh_, :], in1=rB)
                    nc.sync.dma_start(out=ct[j][64:128, ds(cc * CW, CW)],
                                      in_=tmB)

            # ---- warmup: x DMAs + V(0..3) + q(j0) + k(j0) ----------------
            xt_v0, dmas = x_dma_fillers("v", 0)
            for f in dmas:
                f()
            xt_q0, dmas = x_dma_fillers("q", 0)
            for f in dmas:
                f()
            for m in range(4):
                for f in v_proj_fillers(m, xt_v0):
                    f()
            xt_q1, dmas = x_dma_fillers("q", 1)
            for f in dmas:
                f()
            for f in qk_proj_fillers("q", 0, xt_q0, 0):
                f()
            xt_k0, dmas = x_dma_fillers("k", 0)
            for f in dmas:
                f()
            for f in qk_proj_fillers("q", 0, xt_q1, 1):
                f()
            xt_k1, dmas = x_dma_fillers("k", 1)
            for f in dmas:
                f()
            for f in qk_proj_fillers("k", 0, xt_k0, 0):
                f()
            for f in qk_proj_fillers("k", 0, xt_k1, 1):
                f()

            # ---- attention blocks with interleaved projections -----------
            for j in range(J):
                fillers = []
                pre_q = 0
                if j == 0:
                    # finish V: half-0 tiles 4..7, then half 1 (its DMAs wait
                    # on the xv pool buffer, freed by V(7)'s last matmul).
                    xt_v1, dmas = x_dma_fillers("v", 1)
                    fillers += dmas
                    for m in range(4, 8):
                        fillers += v_proj_fillers(m, xt_v0)
                    for m in range(8, 16):
                        fillers += v_proj_fillers(m, xt_v1)
                    # ctx(j0, cc0, m) consumes vt[m]: keep production ahead
                    # of consumption (~1 vt tile + slack per iteration).
                    pre_q = 10
                if j + 1 < J:
                    for which in ("q", "k"):
                        xh0, dmas0 = x_dma_fillers(which, 0)
                        fillers += dmas0
                        fillers += qk_proj_fillers(which, j + 1, xh0, 0)
                        xh1, dmas1 = x_dma_fillers(which, 1)
                        fillers += dmas1
                        fillers += qk_proj_fillers(which, j + 1, xh1, 1)
                attention_block(j, fillers, pre_q)
                while fillers:
                    fillers.pop(0)()

        # ---- phase E: output projection (phase-D PSUM pools closed) ------
        with tc.tile_pool(name="psumO", bufs=2, space="PSUM") as ppo, \
             tc.tile_pool(name="outb", bufs=3) as obpool:
            ndw = 512
            for ft in range(TT):
                po = ppo.tile([P, D_], FP32, tag="po", name="po")
                for j in range(J):
                    for nd in range(D_ // ndw):
                        nc.tensor.matmul(
                            po[:, ts(nd, ndw)],
                            lhsT=ct[j][:, ts(ft, P)],
                            rhs=wo_sb[:, j, ts(nd, ndw)],
                            start=(j == 0), stop=(j == J - 1))
                ob = obpool.tile([P, D_], FP32, tag="ob", name="ob")
                nc.vector.tensor_copy(out=ob, in_=po)
                nc.sync.dma_start(out=out[ts(ft, P), :], in_=ob)

    nc.compile()
    return nc


def shard_inputs(cfg, query_input, key_input, value_input, Wq, Wk, Wv, Wo):
    """Per-core input maps: core c -> batch c//2, head group c%2."""
    hloc = cfg.hloc
    in_maps = []
    for c in range(N_CORES):
        b, g = c // 2, c % 2
        hs = slice(g * hloc, (g + 1) * hloc)
        in_maps.append({
            "xq_t": np.ascontiguousarray(query_input[b].T).astype(BF),
            "xk_t": np.ascontiguousarray(key_input[b].T).astype(BF),
            "xv_t": np.ascontiguousarray(value_input[b].T).astype(BF),
            "wq": np.ascontiguousarray(Wq[:, hs, :]).reshape(cfg.D, cfg.hk).astype(BF),
            "wk": np.ascontiguousarray(Wk[:, hs, :]).reshape(cfg.D, cfg.hk).astype(BF),
            "wv": np.ascontiguousarray(Wv[:, hs, :]).reshape(cfg.D, cfg.hk).astype(BF),
            "wo": np.ascontiguousarray(Wo[hs]).reshape(cfg.hk, cfg.D).astype(BF),
        })
    return in_maps


_nc_cache = {}


def _get_nc(cfg):
    key = (cfg.S, cfg.D, cfg.hloc, cfg.Dh)
    if key not in _nc_cache:
        _nc_cache[key] = build_nc(cfg)
    return _nc_cache[key]


def run_spmd(inputs, trace=False, trace_cores=None):
    """Run the 8-core SPMD kernel; returns (output [B,S,D] fp32, BassKernelResults)."""
    from concourse.bass_utils import run_bass_kernel_spmd

    cfg = Cfg()
    nc = _get_nc(cfg)
    in_maps = shard_inputs(cfg, **{k: np.asarray(v) for k, v in inputs.items()})
    res = run_bass_kernel_spmd(nc, in_maps, list(range(N_CORES)),
                               trace=trace, trace_cores=trace_cores)
    out = np.empty((B, S, D), np.float32)
    for b in range(B):
        out[b] = res.results[2 * b]["out_part"] + res.results[2 * b + 1]["out_part"]
    return out, res


def kernel(**inputs):
    out, _ = run_spmd(inputs)
    return out


# revision 5
# speedup vs baseline: 1.1492x; 1.0832x over previous
"""Multi-head attention (B=4, S=2048, D=1024, H=16, Dh=64) on 8 TRN2 NeuronCores.

Sharding: core c handles batch b = c // 2 and head group g = c % 2 (8 heads
each).  Every core computes Q/K/V projections for its batch+heads, the
attention for those heads, and a *partial* output projection (its heads'
slice of Wo).  The host sums the two partials per batch while unsharding —
the tensor-parallel all-reduce on the output, done during gather.

v3 structure (all bf16 matmul operands, fp32 PSUM accumulation):
  - logits^T[t,f] per head pair via two K=64 matmuls issued back-to-back so
    they run CONCURRENTLY on disjoint PE row groups (rows 0-63 / 64-127).
  - one ScalarE Exp instruction per (m, head-pair): reads the [128, 2, 512]
    logits PSUM tile flat (FD=1024).  ScalarE does nothing else; its ~273us
    exp stream is the phase-D pacing constraint, so every other engine's
    work is scheduled to hide underneath it.
  - pl (logits PSUM) double-buffered so the m+1 logits pair never waits on
    the exp of m; e (exp output) 8-deep so the exp stream rides over the
    per-cc epilogue latency without stalling.
  - vt value tiles are padded to 128 columns with ONES in cols 64..127: the
    ctx matmul then yields ctx^T rows 0..63 plus 64 replicated denominator
    rows 64..127 in one [128, 512] output — full PE width, a 128-wide
    stationary operand (fast-weight-load eligible), free denom "broadcast".
  - softmax epilogue per (j, cc): lane-aligned [64,512] copy of the
    replicated denominator rows, partition-shift DMA to base-0, one
    [64,512] reciprocal_approx_fast, two [64,512] tensor_muls (head B via a
    base-0 temp + partition-shift DMA; DVE lanes are partition-hardwired).
  - everything except a minimal warmup (V tiles 0-3, q window 0, k windows
    0-1 for head pair 0) is emitted as deadline-tagged FILLER closures
    inside the attention m-loops: remaining V tiles and q/k windows for j0,
    then each next head pair's projections, and for j3 the output
    projection of already-finished f-windows.  Deadlines guarantee
    producers are emitted before in-order PE consumers; proportional pacing
    spreads the rest.  x^T inputs stream from DRAM in S/4 quarters
    (re-loaded per head pair; DMA bandwidth has big headroom).
"""

import sys

sys.path.insert(0, "/opt/trn_rl_repo")

import numpy as np
import ml_dtypes

BF = ml_dtypes.bfloat16

# Problem geometry (hardcoded; the harness always calls with these shapes).
B, S, D, H, Dh = 4, 2048, 1024, 16, 64
N_CORES = 8
H_LOC = H // 2          # heads per core
HK = H_LOC * Dh         # 512


class Cfg:
    def __init__(self, S=S, D=D, hloc=H_LOC, Dh=Dh):
        P = 128
        self.S, self.D, self.hloc, self.Dh = S, D, hloc, Dh
        self.P = P
        self.hk = hloc * Dh           # 512
        self.J = hloc // 2            # head pairs = 4
        self.DC = D // P              # contraction chunks = 8
        self.TT = S // P              # t (key) tiles = 16
        self.CW = 512                 # f-chunk width (one PSUM bank)
        self.NCC = S // self.CW       # f-chunks = 4
        self.scale = float(Dh) ** -0.5


def build_nc(cfg):
    import concourse.bass as bass
    import concourse.mybir as mybir
    import concourse.tile as tile
    from concourse import bacc
    from concourse.bass import ds, ts
    from contextlib import ExitStack

    FP32 = mybir.dt.float32
    BF16 = mybir.dt.bfloat16
    EXP = mybir.ActivationFunctionType.Exp

    P, Dh_, hloc = cfg.P, cfg.Dh, cfg.hloc
    S_, D_, hk = cfg.S, cfg.D, cfg.hk
    J, DC, TT, CW, NCC = cfg.J, cfg.DC, cfg.TT, cfg.CW, cfg.NCC
    NQ = S_ // CW                      # x quarters (= 4)

    nc = bacc.Bacc("TRN2")
    xq = nc.declare_dram_parameter("xq_t", [D_, S_], BF16, isOutput=False)
    xk = nc.declare_dram_parameter("xk_t", [D_, S_], BF16, isOutput=False)
    xv = nc.declare_dram_parameter("xv_t", [D_, S_], BF16, isOutput=False)
    wq = nc.declare_dram_parameter("wq", [D_, hk], BF16, isOutput=False)
    wk = nc.declare_dram_parameter("wk", [D_, hk], BF16, isOutput=False)
    wv = nc.declare_dram_parameter("wv", [D_, hk], BF16, isOutput=False)
    wo = nc.declare_dram_parameter("wo", [hk, D_], BF16, isOutput=False)
    out = nc.declare_dram_parameter("out_part", [S_, D_], FP32, isOutput=True)

    xs = {"q": xq, "k": xk, "v": xv}

    with tile.TileContext(nc) as tc, ExitStack() as ctx:
        singles = ctx.enter_context(tc.tile_pool(name="singles", bufs=1))

        # ---- persistent SBUF tensors -------------------------------------
        wq_sb = singles.tile([P, DC, hk], BF16, tag="wq", name="wq")
        wk_sb = singles.tile([P, DC, hk], BF16, tag="wk", name="wk")
        wv_sb = singles.tile([P, DC, hk], BF16, tag="wv", name="wv")
        wo_sb = singles.tile([P, J, D_], BF16, tag="wo", name="wo")
        qT = [singles.tile([P, S_], BF16, tag=f"qT{j}", name=f"qT{j}") for j in range(J)]
        kT = [singles.tile([P, S_], BF16, tag=f"kT{j}", name=f"kT{j}") for j in range(J)]
        ct = [singles.tile([P, S_], BF16, tag=f"ct{j}", name=f"ct{j}") for j in range(J)]
        # vt[m]: [t, head, 128]; cols 0..63 = V, cols 64..127 = 1.0 so the
        # ctx matmul replicates the softmax denominator on rows 64..127.
        vt = [singles.tile([P, hloc, P], BF16, tag=f"vt{m}", name=f"vt{m}")
              for m in range(TT)]

        w_sbs = {"q": wq_sb, "k": wk_sb, "v": wv_sb}

        with tc.tile_pool(name="xv_in", bufs=2) as xvpool, \
             tc.tile_pool(name="xqk_in", bufs=2) as xqkpool, \
             tc.tile_pool(name="psumP", bufs=2, space="PSUM") as pps, \
             tc.tile_pool(name="psumL", bufs=2, space="PSUM") as ppl, \
             tc.tile_pool(name="psumC", bufs=1, space="PSUM") as ppc, \
             tc.tile_pool(name="expp", bufs=8) as epool, \
             tc.tile_pool(name="rbc", bufs=2) as rpool, \
             tc.tile_pool(name="stage", bufs=2) as stpool, \
             tc.tile_pool(name="outb", bufs=3) as obpool:

            def load_w(which):
                w_dram = {"q": wq, "k": wk, "v": wv}[which]
                w_r = w_dram[:, :].rearrange("(a p) n -> p a n", p=P)
                for dc in range(DC):
                    nc.sync.dma_start(out=w_sbs[which][:, dc, :],
                                      in_=w_r[:, dc, :])

            def x_dma_fillers(which, quarter):
                """Closures DMAing one S/4 quarter of an x^T input."""
                pool = xvpool if which == "v" else xqkpool
                xt = pool.tile([P, DC, CW], BF16, tag=f"xt{which != 'v'}",
                               name=f"xt{which}{quarter}")
                src_r = xs[which][:, :].rearrange("(a p) s -> p a s", p=P)

                def mk(dc):
                    def emit():
                        nc.sync.dma_start(
                            out=xt[:, dc, :],
                            in_=src_r[:, dc, ds(quarter * CW, CW)])
                    return emit
                return xt, [mk(dc) for dc in range(DC)]

            def qk_proj_fillers(which, j, xt_q, w):
                """Closures projecting x quarter w -> qT/kT[j] window w."""
                dst = qT[j] if which == "q" else kT[j]
                w_sb = w_sbs[which]
                holder = []

                def mk_mm(dc):
                    def emit():
                        if dc == 0:
                            holder.append(pps.tile([P, CW], FP32,
                                                   tag="psP", name="psP"))
                        nc.tensor.matmul(
                            holder[0],
                            lhsT=w_sb[:, dc, ts(j, P)],
                            rhs=xt_q[:, dc, :],
                            start=(dc == 0), stop=(dc == DC - 1))
                    return emit

                def drain():
                    nc.vector.tensor_copy(out=dst[:, ds(w * CW, CW)],
                                          in_=holder[0])

                return [mk_mm(dc) for dc in range(DC)] + [drain]

            def v_proj_fillers(m, xt_q):
                """Closures producing the ones-padded vt[m] tile."""
                holder = []

                def mk_mm(dc):
                    def emit():
                        if dc == 0:
                            holder.append(pps.tile([P, hk], FP32,
                                                   tag="psP", name="psP"))
                        nc.tensor.matmul(holder[0],
                                         lhsT=xt_q[:, dc, ts(m % 4, P)],
                                         rhs=wv_sb[:, dc, :],
                                         start=(dc == 0), stop=(dc == DC - 1))
                    return emit

                def drain():
                    nc.vector.tensor_copy(
                        out=vt[m][:, :, 0:Dh_],
                        in_=holder[0].rearrange("p (h k) -> p h k", h=hloc))
                    nc.vector.memset(vt[m][:, :, Dh_:P], 1.0)

                return [mk_mm(dc) for dc in range(DC)] + [drain]

            def outproj_filler(ft, nd):
                """One output-projection group: [128,512] over all 4 j."""
                def emit():
                    po = pps.tile([P, CW], FP32, tag="psP", name="psP")
                    for j in range(J):
                        nc.tensor.matmul(
                            po,
                            lhsT=ct[j][:, ts(ft, P)],
                            rhs=wo_sb[:, j, ds(nd * CW, CW)],
                            start=(j == 0), stop=(j == J - 1))
                    ob = obpool.tile([P, CW], FP32, tag="ob", name="ob")
                    nc.vector.tensor_copy(out=ob, in_=po)
                    nc.sync.dma_start(out=out[ts(ft, P), ds(nd * CW, CW)],
                                      in_=ob)
                return emit

            # ---- attention ------------------------------------------------
            def lpair(j, cc, m, pl):
                # two K=64 logits matmuls on disjoint PE row groups
                for hh in range(2):
                    nc.tensor.matmul(
                        pl[:, hh, :],
                        lhsT=kT[j][hh * 64:(hh + 1) * 64, ts(m, P)],
                        rhs=qT[j][hh * 64:(hh + 1) * 64, ds(cc * CW, CW)],
                        start=True, stop=True)

            def attention_block(j, fillers, post_cc=None):
                """One head pair's attention.  fillers: list of
                (deadline_iter, closure), sorted by deadline; items are
                emitted once their deadline arrives or proportional pacing
                calls for them.  post_cc(cc) may append more fillers."""
                n_iter = NCC * TT
                state = {"done": 0, "appended": len(fillers)}

                def drain(it):
                    target = state["appended"] * (it + 1) // n_iter
                    while fillers and (fillers[0][0] <= it
                                       or state["done"] < target):
                        fillers.pop(0)[1]()
                        state["done"] += 1

                it = 0
                for cc in range(NCC):
                    pcA = ppc.tile([P, CW], FP32, tag="pcA", name="pcA")
                    pcB = ppc.tile([P, CW], FP32, tag="pcB", name="pcB")
                    pl = ppl.tile([P, 2, CW], FP32, tag="pl", name="pl")
                    lpair(j, cc, 0, pl)
                    for m in range(TT):
                        e = epool.tile([P, 2, CW], BF16, tag="e", name="e")
                        nc.scalar.activation(out=e, in_=pl, func=EXP,
                                             scale=cfg.scale)
                        if m + 1 < TT:
                            pl = ppl.tile([P, 2, CW], FP32, tag="pl", name="pl")
                            lpair(j, cc, m + 1, pl)
                        for hh in range(2):
                            nc.tensor.matmul(
                                (pcA, pcB)[hh],
                                lhsT=vt[m][:, 2 * j + hh, :],
                                rhs=e[:, hh, :],
                                start=(m == 0), stop=(m == TT - 1))
                        drain(it)
                        it += 1
                    # ---- softmax epilogue for (j, cc) ----------------------
                    stA = stpool.tile([P, CW], FP32, tag="stA", name="stA")
                    nc.vector.tensor_copy(out=stA[64:128, :], in_=pcA[64:128, :])
                    stB = stpool.tile([P, CW], FP32, tag="stB", name="stB")
                    nc.vector.tensor_copy(out=stB[64:128, :], in_=pcB[64:128, :])
                    dA = rpool.tile([64, CW], FP32, tag="dA", name="dA")
                    nc.sync.dma_start(out=dA, in_=stA[64:128, :])
                    dB = rpool.tile([64, CW], FP32, tag="dB", name="dB")
                    nc.sync.dma_start(out=dB, in_=stB[64:128, :])
                    rA = rpool.tile([64, CW], FP32, tag="rA", name="rA")
                    nc.vector.reciprocal_approx_fast(out=rA, in_=dA)
                    rB = rpool.tile([64, CW], FP32, tag="rB", name="rB")
                    nc.vector.reciprocal_approx_fast(out=rB, in_=dB)
                    nc.vector.tensor_mul(out=ct[j][0:64, ds(cc * CW, CW)],
                                         in0=pcA[0:Dh_, :], in1=rA)
                    tmB = stpool.tile([64, CW], BF16, tag="tmB", name="tmB")
                    nc.vector.tensor_mul(out=tmB, in0=pcB[0:Dh_, :], in1=rB)
                    nc.sync.dma_start(out=ct[j][64:128, ds(cc * CW, CW)],
                                      in_=tmB)
                    if post_cc is not None:
                        for item in post_cc(cc):
                            fillers.append(item)
                            state["appended"] += 1
                while fillers:
                    fillers.pop(0)[1]()

            # ---- warmup: just enough for head pair 0 to start ------------
            # DMA order puts the V path first so the PE starts ~immediately.
            # DMAs reusing a ring slot are always emitted AFTER the previous
            # occupant's reads (both here and in filler-list order below).
            xt_vq = [None] * NQ
            xt_vq[0], dmas = x_dma_fillers("v", 0)
            load_w("v")
            for f in dmas:
                f()
            xt_qq = [None] * NQ
            xt_qq[0], dmas = x_dma_fillers("q", 0)
            load_w("q")
            for f in dmas:
                f()
            xt_kq = [None] * NQ
            for m in range(4):
                for f in v_proj_fillers(m, xt_vq[0]):
                    f()
            xt_kq[0], dmas = x_dma_fillers("k", 0)
            load_w("k")
            for f in qk_proj_fillers("q", 0, xt_qq[0], 0):
                f()
            for f in dmas:
                f()
            for f in qk_proj_fillers("k", 0, xt_kq[0], 0):
                f()
            xt_kq[1], dmas = x_dma_fillers("k", 1)
            for f in dmas:
                f()
            nc.sync.dma_start(out=wo_sb,
                              in_=wo[:, :].rearrange("(j p) d -> p j d", p=P))
            for f in qk_proj_fillers("k", 0, xt_kq[1], 1):
                f()

            # ---- j0 fillers: rest of V, rest of q/k(j0), with deadlines.
            # Constraints: V(m) by iter m-1; k window w by iter 4w-2; q
            # window w by iter 16w-2.  Deadlines non-decreasing in list
            # order (the drain pops strictly from the front).
            fl = []
            xt_vq[1], dmas = x_dma_fillers("v", 1)
            fl += [(0, f) for f in dmas]
            fl += [(2, f) for f in v_proj_fillers(4, xt_vq[1])]
            fl += [(3, f) for f in v_proj_fillers(5, xt_vq[1])]
            xt_kq[2], dmas = x_dma_fillers("k", 2)
            fl += [(3, f) for f in dmas]
            fl += [(4, f) for f in qk_proj_fillers("k", 0, xt_kq[2], 2)]
            fl += [(4, f) for f in v_proj_fillers(6, xt_vq[1])]
            fl += [(5, f) for f in v_proj_fillers(7, xt_vq[1])]
            xt_vq[2], dmas = x_dma_fillers("v", 2)
            fl += [(5, f) for f in dmas]
            fl += [(6, f) for f in v_proj_fillers(8, xt_vq[2])]
            fl += [(7, f) for f in v_proj_fillers(9, xt_vq[2])]
            xt_kq[3], dmas = x_dma_fillers("k", 3)
            fl += [(7, f) for f in dmas]
            fl += [(8, f) for f in qk_proj_fillers("k", 0, xt_kq[3], 3)]
            fl += [(8, f) for f in v_proj_fillers(10, xt_vq[2])]
            fl += [(9, f) for f in v_proj_fillers(11, xt_vq[2])]
            xt_vq[3], dmas = x_dma_fillers("v", 3)
            fl += [(9, f) for f in dmas]
            fl += [(10, f) for f in v_proj_fillers(12, xt_vq[3])]
            fl += [(11, f) for f in v_proj_fillers(13, xt_vq[3])]
            xt_qq[1], dmas = x_dma_fillers("q", 1)
            fl += [(11, f) for f in dmas]
            fl += [(12, f) for f in v_proj_fillers(14, xt_vq[3])]
            fl += [(12, f) for f in qk_proj_fillers("q", 0, xt_qq[1], 1)]
            fl += [(13, f) for f in v_proj_fillers(15, xt_vq[3])]
            xt_qq[2], dmas = x_dma_fillers("q", 2)
            fl += [(27, f) for f in dmas]
            fl += [(28, f) for f in qk_proj_fillers("q", 0, xt_qq[2], 2)]
            xt_qq[3], dmas = x_dma_fillers("q", 3)
            fl += [(43, f) for f in dmas]
            fl += [(44, f) for f in qk_proj_fillers("q", 0, xt_qq[3], 3)]

            n_iter = NCC * TT
            for j in range(J):
                if j + 1 < J:
                    # next head pair's projections: due any time this block
                    for which in ("q", "k"):
                        for w in range(NQ):
                            xt_w, dmas = x_dma_fillers(which, w)
                            fl += [(n_iter - 1, f) for f in dmas]
                            fl += [(n_iter - 1, f) for f in
                                   qk_proj_fillers(which, j + 1, xt_w, w)]
                    post_cc = None
                else:
                    # j3: output projection of finished f-windows as fillers
                    def post_cc(cc):
                        return [(n_iter - 1, outproj_filler(ft, nd))
                                for ft in range(4 * cc, 4 * cc + 4)
                                for nd in range(D_ // CW)]
                attention_block(j, fl, post_cc)
                fl = []

    nc.compile()
    return nc


def shard_inputs(cfg, query_input, key_input, value_input, Wq, Wk, Wv, Wo):
    """Per-core input maps: core c -> batch c//2, head group c%2."""
    hloc = cfg.hloc
    in_maps = []
    for c in range(N_CORES):
        b, g = c // 2, c % 2
        hs = slice(g * hloc, (g + 1) * hloc)
        in_maps.append({
            "xq_t": np.ascontiguousarray(query_input[b].T).astype(BF),
            "xk_t": np.ascontiguousarray(key_input[b].T).astype(BF),
            "xv_t": np.ascontiguousarray(value_input[b].T).astype(BF),
            "wq": np.ascontiguousarray(Wq[:, hs, :]).reshape(cfg.D, cfg.hk).astype(BF),
            "wk": np.ascontiguousarray(Wk[:, hs, :]).reshape(cfg.D, cfg.hk).astype(BF),
            "wv": np.ascontiguousarray(Wv[:, hs, :]).reshape(cfg.D, cfg.hk).astype(BF),
            "wo": np.ascontiguousarray(Wo[hs]).reshape(cfg.hk, cfg.D).astype(BF),
        })
    return in_maps


_nc_cache = {}


def _get_nc(cfg):
    key = (cfg.S, cfg.D, cfg.hloc, cfg.Dh)
    if key not in _nc_cache:
        _nc_cache[key] = build_nc(cfg)
    return _nc_cache[key]


def run_spmd(inputs, trace=False, trace_cores=None):
    """Run the 8-core SPMD kernel; returns (output [B,S,D] fp32, BassKernelResults)."""
    from concourse.bass_utils import run_bass_kernel_spmd

    cfg = Cfg()
    nc = _get_nc(cfg)
    in_maps = shard_inputs(cfg, **{k: np.asarray(v) for k, v in inputs.items()})
    res = run_bass_kernel_spmd(nc, in_maps, list(range(N_CORES)),
                               trace=trace, trace_cores=trace_cores)
    out = np.empty((B, S, D), np.float32)
    for b in range(B):
        out[b] = res.results[2 * b]["out_part"] + res.results[2 * b + 1]["out_part"]
    return out, res


def kernel(**inputs):
    out, _ = run_spmd(inputs)
    return out
